# revision 8
# baseline (speedup 1.0000x reference)
"""Trainium2 Bass kernel for nn_BoundaryLoss2 (dice + BCE + boundary loss).

Strategy (data-parallel over batch, one sample per core, 8 cores):
  The expensive part is the exact euclidean distance transform (EDT) of the
  target mask (and its complement) per sample.  The reference computes
      d2[i,j] = min_j' ( g[i,j']^2 + (j-j')^2 ),   g = vertical L1 distance
  For 50%-density random masks the true distances are tiny (max d2 == 9 for
  the actual inputs), so a windowed min-plus with radius K is *exact* as long
  as max(d2) <= K^2; this condition is verified on-device (max-reduce of the
  computed field) and a host-side numpy fallback guarantees correctness
  otherwise.

  Per core pipeline:
    - load logits/targets (256x256) in row-layout [128 part, 2 group, 256 col]
    - build mask cost maps c in {0, BIG} (bf16), PE-transpose to column-layout
    - vertical L1 distance with two tensor_tensor_scan passes (fwd + reversed)
    - PE-transpose back, squaring fused into the PSUM->SBUF copy (ScalarE)
    - windowed parabola pass: acc = min_d ( g2(col+d) + d^2 ), |d| <= K,
      via tensor_tensor(min) + fused scalar_tensor_tensor(add, min)
    - d1 - d0 is the signed distance term (d1 zero on positives, d0 zero on
      negatives), boundary sum = sum(sig*(d1-d0)) + sum(sig*t)
    - all loss terms reduce to per-partition partial sums -> [128, 8] output
  Host gathers the 8 [128,8] stats tensors and combines the scalars.
"""

import numpy as np

import concourse.bacc as bacc
import concourse.bass as bass
import concourse.tile as tile
from concourse import mybir
from concourse.bass_utils import run_bass_kernel_spmd
from concourse.masks import make_identity

P = 128
H = 256
W = 256
NCORES = 8
B = 8
K = 4  # window radius; result exact iff max(d2) <= K*K (checked on device)
BIG = 30000.0
GAP = 8  # border gap in the parabola tile (>= K, 8 keeps alignment)
SMOOTH = 1e-5
F32 = mybir.dt.float32
BF16 = mybir.dt.bfloat16

# stats column layout
S_SIG, S_T, S_LT, S_ST, S_SP, S_SD1, S_MAXW2, S_SD0 = range(8)


def build_boundary_loss_core(tc, stats_out, logits_in, targets_in,
                             use_softplus=False, pool_scans=False):
    """Emit the per-core kernel. APs: stats_out [P,8] f32 (DRAM out),
    logits_in/targets_in [H,W] f32 (DRAM in)."""
    nc = tc.nc
    Alu = mybir.AluOpType
    Act = mybir.ActivationFunctionType
    WP = W + 2 * GAP  # padded parabola row width
    WR = WP - 8  # parabola op region width (reads stay in bounds for |d|<=4)

    with (
        tc.tile_pool(name="consts", bufs=1) as consts,
        tc.tile_pool(name="work", bufs=1) as work,
        tc.tile_pool(name="psum", bufs=4, space=bass.MemorySpace.PSUM) as psum,
    ):
        ident = consts.tile([P, P], BF16)
        make_identity(nc, ident)
        ones_h = consts.tile([P, H], BF16)
        nc.gpsimd.memset(ones_h, 1.0)
        bias0 = consts.tile([P, 1], F32)
        nc.gpsimd.memset(bias0, 0.0)
        bias1 = consts.tile([P, 1], F32)
        nc.gpsimd.memset(bias1, 1.0)

        # ---- load inputs in row layout [p, g, j] = img[g*128+p, j] ----
        t_src = targets_in.rearrange("(g p) w -> p g w", p=P)
        l_src = logits_in.rearrange("(g p) w -> p g w", p=P)
        t_b = work.tile([P, 2, W], F32)
        nc.gpsimd.dma_start(out=t_b[:, 0], in_=t_src[:, 0])
        nc.sync.dma_start(out=t_b[:, 1], in_=t_src[:, 1])
        l_b = work.tile([P, 2, W], F32)
        nc.sync.dma_start(out=l_b, in_=l_src)

        # ---- mask cost maps: c[m] = 0 where (t == m-target) else BIG ----
        # m=0: positives mask (t==1 -> 0), m=1: negatives mask (t==0 -> 0)
        c_b = work.tile([P, 2, 2, W], BF16)  # [p, m, g, j]
        nc.vector.tensor_scalar(c_b[:, 0], t_b, -BIG, BIG, op0=Alu.mult, op1=Alu.add)
        nc.gpsimd.tensor_scalar(c_b[:, 1], t_b, BIG, None, op0=Alu.mult)

        # ---- transpose to column layout [p, m, cg, i] = c[m][i, cg*128+p] ----
        cA = work.tile([P, 2, 2, H], BF16)
        for m in range(2):
            for g in range(2):
                for cg in range(2):
                    pt = psum.tile([P, P], BF16)
                    nc.tensor.transpose(pt, c_b[:, m, g, cg * P:(cg + 1) * P], ident)
                    nc.vector.tensor_copy(cA[:, m, cg, g * P:(g + 1) * P], pt)

        # ---- vertical L1 distance: two scans (fwd, then bwd over fwd) ----
        ft = work.tile([P, 2, 2, H], BF16)
        gt = work.tile([P, 2, 2, H], BF16)
        for m in range(2):
            eng = nc.gpsimd if (pool_scans and m == 1) else nc.vector
            for cg in range(2):
                eng.tensor_tensor_scan(
                    ft[:, m, cg], ones_h, cA[:, m, cg], BIG, Alu.add, Alu.min)
                eng.tensor_tensor_scan(
                    gt[:, m, cg][:, ::-1], ones_h, ft[:, m, cg][:, ::-1],
                    BIG, Alu.add, Alu.min)

        # ---- transpose back to row layout, squaring fused into the copy ----
        g2b = work.tile([P, 2, 2, WP], BF16)  # [p, m, g, GAP+j]
        nc.gpsimd.memset(g2b[:, :, :, 0:GAP], BIG)
        nc.gpsimd.memset(g2b[:, :, :, GAP + W:], BIG)
        for m in range(2):
            for cg in range(2):
                for g in range(2):
                    pt = psum.tile([P, P], BF16)
                    nc.tensor.transpose(pt, gt[:, m, cg, g * P:(g + 1) * P], ident)
                    nc.scalar.activation(
                        g2b[:, m, g, GAP + cg * P:GAP + (cg + 1) * P], pt, Act.Square,
                        bias=bias0)

        # ---- windowed parabola pass along columns ----
        def sh(d):
            return g2b[:, :, :, 4 + d:4 + d + WR]

        acc = work.tile([P, 2, 2, WR], BF16)
        for d in range(1, K + 1):
            u = work.tile([P, 2, 2, WR], BF16, name=f"u{d}")
            nc.vector.tensor_tensor(u, sh(-d), sh(d), Alu.min)
            nc.gpsimd.tensor_scalar(u, u, float(d * d), None, op0=Alu.add)
            if d == 1:
                nc.vector.tensor_tensor(acc, sh(0), u, Alu.min)
            else:
                nc.vector.tensor_tensor(acc, acc, u, Alu.min)

        # interior view: image col j lives at acc[..., 4 + j]
        w2 = acc[:, :, :, 4:4 + W]  # [p, m, g, j]

        stats = work.tile([P, 8], F32)
        nc.vector.memset(stats, 0.0)
        nc.vector.tensor_reduce(
            stats[:, S_MAXW2:S_MAXW2 + 1], w2, axis=mybir.AxisListType.XYZ,
            op=Alu.max)

        # ---- distances and loss terms ----
        dst = work.tile([P, 2, 2, W], F32)  # [p, m, g, j]; m=0 -> d1, m=1 -> d0
        nc.scalar.activation(dst, w2, Act.Sqrt, bias=bias0)

        sig = work.tile([P, 2, W], F32)
        nc.scalar.activation(
            sig, l_b, Act.Sigmoid, bias=bias0, accum_out=stats[:, S_SIG:S_SIG + 1])
        sp = work.tile([P, 2, W], F32)
        if use_softplus:
            nc.scalar.activation(
                sp, l_b, Act.Softplus, bias=bias0,
                accum_out=stats[:, S_SP:S_SP + 1])
        else:  # simulator lacks Softplus: ln(1 + exp(l))
            ex = work.tile([P, 2, W], F32)
            nc.scalar.activation(ex, l_b, Act.Exp, bias=bias0)
            nc.scalar.activation(
                sp, ex, Act.Ln, bias=bias1, accum_out=stats[:, S_SP:S_SP + 1])

        tsum = work.tile([P, 2, W], F32)
        nc.vector.tensor_scalar(
            tsum, t_b, 1.0, None, op0=Alu.mult, op1=Alu.add,
            accum_out=stats[:, S_T:S_T + 1])

        lt = work.tile([P, 2, W], F32)
        nc.vector.scalar_tensor_tensor(
            lt, l_b, 1.0, t_b, op0=Alu.mult, op1=Alu.mult,
            accum_out=stats[:, S_LT:S_LT + 1])
        st = work.tile([P, 2, W], F32)
        nc.vector.scalar_tensor_tensor(
            st, sig, 1.0, t_b, op0=Alu.mult, op1=Alu.mult,
            accum_out=stats[:, S_ST:S_ST + 1])
        sd1 = work.tile([P, 2, W], F32)
        nc.vector.scalar_tensor_tensor(
            sd1, sig, 1.0, dst[:, 0], op0=Alu.mult, op1=Alu.mult,
            accum_out=stats[:, S_SD1:S_SD1 + 1])
        sd0 = work.tile([P, 2, W], F32)
        nc.vector.scalar_tensor_tensor(
            sd0, sig, 1.0, dst[:, 1], op0=Alu.mult, op1=Alu.mult,
            accum_out=stats[:, S_SD0:S_SD0 + 1])

        nc.sync.dma_start(out=stats_out, in_=stats)


_CACHE = {}


def _get_nc():
    if "nc" not in _CACHE:
        nc = bacc.Bacc("TRN2", target_bir_lowering=False, debug=False)
        logits_in = nc.dram_tensor("logits", (H, W), F32, kind="ExternalInput").ap()
        targets_in = nc.dram_tensor("targets", (H, W), F32, kind="ExternalInput").ap()
        stats_out = nc.dram_tensor("stats", (P, 8), F32, kind="ExternalOutput").ap()
        with tile.TileContext(nc) as tc:
            build_boundary_loss_core(tc, stats_out, logits_in, targets_in)
        nc.compile()
        _CACHE["nc"] = nc
    return _CACHE["nc"]


def combine_stats(stats):
    """stats: (NCORES, P, 8) -> scalar loss (np.float32). None if the
    windowed EDT was not provably exact (caller must fall back)."""
    maxw2 = float(stats[:, :, S_MAXW2].max())
    if maxw2 > K * K + 0.5:
        return None
    s = stats.sum(axis=1, dtype=np.float64)  # (NCORES, 8)
    n = float(B * H * W)
    s_sig, s_t = s[:, S_SIG], s[:, S_T]
    s_lt, s_st = s[:, S_LT], s[:, S_ST]
    s_sp = s[:, S_SP]
    s_sdq = s[:, S_SD1] - s[:, S_SD0]
    has_pos = s_t > 0
    inter = s_st.sum()
    union = s_sig.sum() + s_t.sum() + SMOOTH
    dice = 1.0 - (2.0 * inter + SMOOTH) / union
    bce = (s_sp.sum() - s_lt.sum()) / n
    bdy = np.where(has_pos, s_sdq + s_st, 0.0).sum() / n
    return np.float32(0.5 * dice + 0.5 * bce + 0.5 * bdy)


def run_device(logits, targets, trace=False, trace_cores=None):
    l = np.ascontiguousarray(np.asarray(logits, np.float32).reshape(NCORES, H, W))
    t = np.ascontiguousarray(np.asarray(targets, np.float32).reshape(NCORES, H, W))
    in_maps = [{"logits": l[i], "targets": t[i]} for i in range(NCORES)]
    nc = _get_nc()
    res = run_bass_kernel_spmd(
        nc, in_maps, core_ids=list(range(NCORES)), trace=trace,
        trace_cores=trace_cores)
    stats = np.stack([res.results[i]["stats"] for i in range(NCORES)])
    return stats, res


# ---------------- host fallback (exact reference semantics) ----------------

def _edt_np(mask):
    """Exact EDT (distance to nearest True) matching the reference."""
    h, w = mask.shape
    big = float(h * w)
    c = np.where(mask, 0.0, np.inf)
    f = np.empty((h, w))
    s = np.full((w,), big)
    for i in range(h):
        s = np.minimum(s + 1.0, c[i])
        f[i] = s
    g = np.empty((h, w))
    s = np.full((w,), big)
    for i in reversed(range(h)):
        s = np.minimum(s + 1.0, f[i])
        g[i] = s
    g2 = g * g
    jj = np.arange(w, dtype=np.float64)
    dj2 = (jj[:, None] - jj[None, :]) ** 2  # (j_out, j_src)
    d2 = np.empty((h, w))
    for i in range(h):
        d2[i] = (g2[i][None, :] + dj2).min(axis=1)
    return np.sqrt(d2)


def _fallback_loss(logits, targets):
    l = np.asarray(logits, np.float64).reshape(B, H, W)
    t = np.asarray(targets, np.float64).reshape(B, H, W)
    sig = 1.0 / (1.0 + np.exp(-l))
    inter = (sig * t).sum()
    union = sig.sum() + t.sum() + SMOOTH
    dice = 1.0 - (2.0 * inter + SMOOTH) / union
    bce = (np.logaddexp(l, 0.0) - l * t).mean()
    bdy_sum = 0.0
    for b_i in range(B):
        m = t[b_i] > 0.5
        if not m.any():
            continue
        d1 = _edt_np(m)
        d0 = _edt_np(~m)
        res = d1 * (1.0 - t[b_i]) - (d0 - 1.0) * t[b_i]
        bdy_sum += (sig[b_i] * res).sum()
    bdy = bdy_sum / float(B * H * W)
    return np.float32(0.5 * dice + 0.5 * bce + 0.5 * bdy)


def kernel(logits, targets):
    stats, _ = run_device(logits, targets)
    loss = combine_stats(stats)
    if loss is None:
        loss = _fallback_loss(logits, targets)
    return np.array(loss, dtype=np.float32)


# revision 9
# speedup vs baseline: 2.9301x; 2.9301x over previous
"""Trainium2 Bass kernel for nn_BoundaryLoss2 (dice + BCE + boundary loss).

Strategy (data-parallel over batch, one sample per core, 8 cores):
  The expensive part is the exact euclidean distance transform (EDT) of the
  target mask (and its complement) per sample.  The reference computes
      d2[i,j] = min_j' ( g[i,j']^2 + (j-j')^2 ),   g = vertical L1 distance
  For 50%-density random masks the true distances are tiny (max d2 == 9 for
  the actual inputs), so a windowed min-plus with radius K is *exact* as long
  as max(d2) <= K^2; this condition is verified on-device (max-reduce of the
  computed field) and a host-side numpy fallback guarantees correctness
  otherwise.

  Per core pipeline:
    - load logits/targets (256x256) in row-layout [128 part, 2 group, 256 col]
    - build mask cost maps c in {0, BIG} (bf16), PE-transpose to column-layout
    - vertical L1 distance with two tensor_tensor_scan passes (fwd + reversed)
    - PE-transpose back, squaring fused into the PSUM->SBUF copy (ScalarE)
    - windowed parabola pass: acc = min_d ( g2(col+d) + d^2 ), |d| <= K,
      via tensor_tensor(min) + fused scalar_tensor_tensor(add, min)
    - d1 - d0 is the signed distance term (d1 zero on positives, d0 zero on
      negatives), boundary sum = sum(sig*(d1-d0)) + sum(sig*t)
    - all loss terms reduce to per-partition partial sums -> [128, 8] output
  Host gathers the 8 [128,8] stats tensors and combines the scalars.
"""

import numpy as np

import concourse.bacc as bacc
import concourse.bass as bass
import concourse.tile as tile
from concourse import mybir
from concourse.bass_utils import run_bass_kernel_spmd
from concourse.masks import make_identity

P = 128
H = 256
W = 256
NCORES = 8
B = 8
K = 4  # window radius; result exact iff max(d2) <= K*K (checked on device)
BIG = 30000.0
GAP = 8  # border gap in the parabola tile (>= K, 8 keeps alignment)
SMOOTH = 1e-5
F32 = mybir.dt.float32
BF16 = mybir.dt.bfloat16

# stats column layout
S_SIG, S_T, S_LT, S_ST, S_SP, S_SD1, S_MAXW2, S_SD0 = range(8)


def build_boundary_loss_core(tc, stats_out, logits_in, targets_in,
                             use_softplus=False, pool_scans=False):
    """Emit the per-core kernel. APs: stats_out [P,8] f32 (DRAM out),
    logits_in/targets_in [H,W] f32 (DRAM in)."""
    nc = tc.nc
    Alu = mybir.AluOpType
    Act = mybir.ActivationFunctionType
    WP = W + 2 * GAP  # padded parabola row width
    WR = WP - 8  # parabola op region width (reads stay in bounds for |d|<=4)

    with (
        tc.tile_pool(name="consts", bufs=1) as consts,
        tc.tile_pool(name="work", bufs=1) as work,
        tc.tile_pool(name="psum", bufs=4, space=bass.MemorySpace.PSUM) as psum,
    ):
        ident = consts.tile([P, P], BF16)
        make_identity(nc, ident)
        ones_h = consts.tile([P, H], BF16)
        nc.gpsimd.memset(ones_h, 1.0)
        bias0 = consts.tile([P, 1], F32)
        nc.gpsimd.memset(bias0, 0.0)
        bias1 = consts.tile([P, 1], F32)
        nc.gpsimd.memset(bias1, 1.0)
        bias_d2 = []
        for d in range(1, K + 1):
            bt = consts.tile([P, 1], F32, name=f"bias_d2_{d}")
            nc.gpsimd.memset(bt, float(d * d))
            bias_d2.append(bt)

        # ---- load inputs in row layout [p, g, j] = img[g*128+p, j] ----
        t_src = targets_in.rearrange("(g p) w -> p g w", p=P)
        l_src = logits_in.rearrange("(g p) w -> p g w", p=P)
        t_b = work.tile([P, 2, W], F32)
        nc.gpsimd.dma_start(out=t_b[:, 0], in_=t_src[:, 0])
        nc.sync.dma_start(out=t_b[:, 1], in_=t_src[:, 1])
        l_b = work.tile([P, 2, W], F32)
        nc.sync.dma_start(out=l_b, in_=l_src)

        # ---- mask cost maps: c[m] = 0 where (t == m-target) else BIG ----
        # m=0: positives mask (t==1 -> 0), m=1: negatives mask (t==0 -> 0)
        c_b = work.tile([P, 2, 2, W], BF16)  # [p, m, g, j]
        nc.vector.tensor_scalar(c_b[:, 0], t_b, -BIG, BIG, op0=Alu.mult, op1=Alu.add)
        nc.vector.tensor_scalar(c_b[:, 1], t_b, BIG, None, op0=Alu.mult)

        # ---- transpose to column layout [p, m, cg, i] = c[m][i, cg*128+p] ----
        cA = work.tile([P, 2, 2, H], BF16)
        for m in range(2):
            for g in range(2):
                for cg in range(2):
                    pt = psum.tile([P, P], BF16)
                    nc.tensor.transpose(pt, c_b[:, m, g, cg * P:(cg + 1) * P], ident)
                    nc.vector.tensor_copy(cA[:, m, cg, g * P:(g + 1) * P], pt)

        # ---- vertical L1 distance: two scans (fwd, then bwd over fwd) ----
        ft = work.tile([P, 2, 2, H], BF16)
        gt = work.tile([P, 2, 2, H], BF16)
        for m in range(2):
            eng = nc.gpsimd if (pool_scans and m == 1) else nc.vector
            for cg in range(2):
                eng.tensor_tensor_scan(
                    ft[:, m, cg], ones_h, cA[:, m, cg], BIG, Alu.add, Alu.min)
                eng.tensor_tensor_scan(
                    gt[:, m, cg][:, ::-1], ones_h, ft[:, m, cg][:, ::-1],
                    BIG, Alu.add, Alu.min)

        # ---- transpose back to row layout, squaring fused into the copy ----
        g2b = work.tile([P, 2, 2, WP], BF16)  # [p, m, g, GAP+j]
        nc.gpsimd.memset(g2b[:, :, :, 0:GAP], BIG)
        nc.gpsimd.memset(g2b[:, :, :, GAP + W:], BIG)
        for m in range(2):
            for cg in range(2):
                for g in range(2):
                    pt = psum.tile([P, P], BF16)
                    nc.tensor.transpose(pt, gt[:, m, cg, g * P:(g + 1) * P], ident)
                    nc.scalar.activation(
                        g2b[:, m, g, GAP + cg * P:GAP + (cg + 1) * P], pt, Act.Square,
                        bias=bias0)

        # ---- windowed parabola pass along columns ----
        def sh(d):
            return g2b[:, :, :, 4 + d:4 + d + WR]

        acc = work.tile([P, 2, 2, WR], BF16)
        for d in range(1, K + 1):
            u = work.tile([P, 2, 2, WR], BF16, name=f"u{d}")
            nc.vector.tensor_tensor(u, sh(-d), sh(d), Alu.min)
            nc.scalar.activation(u, u, Act.Identity, bias=bias_d2[d - 1])
            if d == 1:
                nc.vector.tensor_tensor(acc, sh(0), u, Alu.min)
            else:
                nc.vector.tensor_tensor(acc, acc, u, Alu.min)

        # interior view: image col j lives at acc[..., 4 + j]
        w2 = acc[:, :, :, 4:4 + W]  # [p, m, g, j]

        stats = work.tile([P, 8], F32)
        nc.vector.memset(stats, 0.0)
        nc.vector.tensor_reduce(
            stats[:, S_MAXW2:S_MAXW2 + 1], w2, axis=mybir.AxisListType.XYZ,
            op=Alu.max)

        # ---- distances and loss terms ----
        dst = work.tile([P, 2, 2, W], F32)  # [p, m, g, j]; m=0 -> d1, m=1 -> d0
        nc.scalar.activation(dst, w2, Act.Sqrt, bias=bias0)

        sig = work.tile([P, 2, W], F32)
        nc.scalar.activation(
            sig, l_b, Act.Sigmoid, bias=bias0, accum_out=stats[:, S_SIG:S_SIG + 1])
        sp = work.tile([P, 2, W], F32)
        if use_softplus:
            nc.scalar.activation(
                sp, l_b, Act.Softplus, bias=bias0,
                accum_out=stats[:, S_SP:S_SP + 1])
        else:  # simulator lacks Softplus: ln(1 + exp(l))
            ex = work.tile([P, 2, W], F32)
            nc.scalar.activation(ex, l_b, Act.Exp, bias=bias0)
            nc.scalar.activation(
                sp, ex, Act.Ln, bias=bias1, accum_out=stats[:, S_SP:S_SP + 1])

        tsum = work.tile([P, 2, W], F32)
        nc.vector.tensor_scalar(
            tsum, t_b, 1.0, None, op0=Alu.mult, op1=Alu.add,
            accum_out=stats[:, S_T:S_T + 1])

        lt = work.tile([P, 2, W], F32)
        nc.vector.scalar_tensor_tensor(
            lt, l_b, 1.0, t_b, op0=Alu.mult, op1=Alu.mult,
            accum_out=stats[:, S_LT:S_LT + 1])
        st = work.tile([P, 2, W], F32)
        nc.vector.scalar_tensor_tensor(
            st, sig, 1.0, t_b, op0=Alu.mult, op1=Alu.mult,
            accum_out=stats[:, S_ST:S_ST + 1])
        sd1 = work.tile([P, 2, W], F32)
        nc.vector.scalar_tensor_tensor(
            sd1, sig, 1.0, dst[:, 0], op0=Alu.mult, op1=Alu.mult,
            accum_out=stats[:, S_SD1:S_SD1 + 1])
        sd0 = work.tile([P, 2, W], F32)
        nc.vector.scalar_tensor_tensor(
            sd0, sig, 1.0, dst[:, 1], op0=Alu.mult, op1=Alu.mult,
            accum_out=stats[:, S_SD0:S_SD0 + 1])

        nc.sync.dma_start(out=stats_out, in_=stats)


_CACHE = {}


def _get_nc():
    if "nc" not in _CACHE:
        nc = bacc.Bacc("TRN2", target_bir_lowering=False, debug=False)
        logits_in = nc.dram_tensor("logits", (H, W), F32, kind="ExternalInput").ap()
        targets_in = nc.dram_tensor("targets", (H, W), F32, kind="ExternalInput").ap()
        stats_out = nc.dram_tensor("stats", (P, 8), F32, kind="ExternalOutput").ap()
        with tile.TileContext(nc) as tc:
            build_boundary_loss_core(tc, stats_out, logits_in, targets_in)
        nc.compile()
        _CACHE["nc"] = nc
    return _CACHE["nc"]


def combine_stats(stats):
    """stats: (NCORES, P, 8) -> scalar loss (np.float32). None if the
    windowed EDT was not provably exact (caller must fall back)."""
    maxw2 = float(stats[:, :, S_MAXW2].max())
    if maxw2 > K * K + 0.5:
        return None
    s = stats.sum(axis=1, dtype=np.float64)  # (NCORES, 8)
    n = float(B * H * W)
    s_sig, s_t = s[:, S_SIG], s[:, S_T]
    s_lt, s_st = s[:, S_LT], s[:, S_ST]
    s_sp = s[:, S_SP]
    s_sdq = s[:, S_SD1] - s[:, S_SD0]
    has_pos = s_t > 0
    inter = s_st.sum()
    union = s_sig.sum() + s_t.sum() + SMOOTH
    dice = 1.0 - (2.0 * inter + SMOOTH) / union
    bce = (s_sp.sum() - s_lt.sum()) / n
    bdy = np.where(has_pos, s_sdq + s_st, 0.0).sum() / n
    return np.float32(0.5 * dice + 0.5 * bce + 0.5 * bdy)


def run_device(logits, targets, trace=False, trace_cores=None):
    l = np.ascontiguousarray(np.asarray(logits, np.float32).reshape(NCORES, H, W))
    t = np.ascontiguousarray(np.asarray(targets, np.float32).reshape(NCORES, H, W))
    in_maps = [{"logits": l[i], "targets": t[i]} for i in range(NCORES)]
    nc = _get_nc()
    res = run_bass_kernel_spmd(
        nc, in_maps, core_ids=list(range(NCORES)), trace=trace,
        trace_cores=trace_cores)
    stats = np.stack([res.results[i]["stats"] for i in range(NCORES)])
    return stats, res


# ---------------- host fallback (exact reference semantics) ----------------

def _edt_np(mask):
    """Exact EDT (distance to nearest True) matching the reference."""
    h, w = mask.shape
    big = float(h * w)
    c = np.where(mask, 0.0, np.inf)
    f = np.empty((h, w))
    s = np.full((w,), big)
    for i in range(h):
        s = np.minimum(s + 1.0, c[i])
        f[i] = s
    g = np.empty((h, w))
    s = np.full((w,), big)
    for i in reversed(range(h)):
        s = np.minimum(s + 1.0, f[i])
        g[i] = s
    g2 = g * g
    jj = np.arange(w, dtype=np.float64)
    dj2 = (jj[:, None] - jj[None, :]) ** 2  # (j_out, j_src)
    d2 = np.empty((h, w))
    for i in range(h):
        d2[i] = (g2[i][None, :] + dj2).min(axis=1)
    return np.sqrt(d2)


def _fallback_loss(logits, targets):
    l = np.asarray(logits, np.float64).reshape(B, H, W)
    t = np.asarray(targets, np.float64).reshape(B, H, W)
    sig = 1.0 / (1.0 + np.exp(-l))
    inter = (sig * t).sum()
    union = sig.sum() + t.sum() + SMOOTH
    dice = 1.0 - (2.0 * inter + SMOOTH) / union
    bce = (np.logaddexp(l, 0.0) - l * t).mean()
    bdy_sum = 0.0
    for b_i in range(B):
        m = t[b_i] > 0.5
        if not m.any():
            continue
        d1 = _edt_np(m)
        d0 = _edt_np(~m)
        res = d1 * (1.0 - t[b_i]) - (d0 - 1.0) * t[b_i]
        bdy_sum += (sig[b_i] * res).sum()
    bdy = bdy_sum / float(B * H * W)
    return np.float32(0.5 * dice + 0.5 * bce + 0.5 * bdy)


def kernel(logits, targets):
    stats, _ = run_device(logits, targets)
    loss = combine_stats(stats)
    if loss is None:
        loss = _fallback_loss(logits, targets)
    return np.array(loss, dtype=np.float32)


# revision 12
# speedup vs baseline: 3.0008x; 1.0241x over previous
"""Trainium2 Bass kernel for nn_BoundaryLoss2 (dice + BCE + boundary loss).

Strategy (data-parallel over batch, one sample per core, 8 cores):
  The expensive part is the exact euclidean distance transform (EDT) of the
  target mask (and its complement) per sample:
      d2[i,j] = min_{di,dj} ( di^2 + dj^2 : mask[i+di, j+dj] )
  decomposed separably into a vertical pass (g = vertical L1 distance) and a
  horizontal parabola pass  w2[i,j] = min_dj ( g[i,j+dj]^2 + dj^2 ).

  Vertical pass runs on the (otherwise idle) tensor engine as a band matmul
      S[i,j] = sum_i' 4^(-|i-i'|) * mask[i',j]
  Since at most two mask pixels exist per distance, S in [4^-g, 8/3*4^-g), so
  g = ceil((127 - exponent(S)) / 2) = (128 - exponent(S)) >> 1   exactly.
  The exponent is extracted with integer ops on the DVE and squared on the
  scalar engine directly into the padded parabola tile.

  Horizontal pass is a windowed min-plus over shifts |dj| <= K executed as
  tensor_tensor(min) + add(d^2) + tensor_tensor(min) chains on DVE + ACT.
  The windowed result is *exact* iff max(w2) <= K^2, verified on device via a
  max-reduce; a host numpy fallback guarantees correctness otherwise (never
  taken for 50%-density random masks, whose true max d2 is ~9).

  d1 (distance to positives) is 0 on positives and d0 (to negatives) is 0 on
  negatives, so the reference's signed map res = d1*(1-t) - (d0-1)*t satisfies
  sig*res = sig*d1 - sig*d0 + sig*t summed per sample.  All loss terms reduce
  to per-partition partial sums -> [128, 8] per-core output, combined on host.
"""

import numpy as np
import ml_dtypes

import concourse.bacc as bacc
import concourse.bass as bass
import concourse.tile as tile
from concourse import mybir
from concourse.bass_utils import run_bass_kernel_spmd

P = 128
H = 256
W = 256
NCORES = 8
B = 8
K = 4  # window radius; result exact iff max(d2) <= K*K (checked on device)
BIG = 30000.0
GAP = 8  # border gap in the parabola tile (>= K, 8 keeps alignment)
SMOOTH = 1e-5
F32 = mybir.dt.float32
BF16 = mybir.dt.bfloat16
I32 = mybir.dt.int32
U32 = mybir.dt.uint32

# stats column layout
S_SIG, S_T, S_LT, S_ST, S_SP, S_SD1, S_MAXW2, S_SD0 = range(8)


def make_wband():
    """[3,128,128] bf16 band-weight blocks: 0=diag 4^-|k-m|, 1=up (out group
    above src group), 2=down. Exact powers of four (exponent-only in bf16)."""
    k = np.arange(P)
    d_diag = np.abs(k[:, None] - k[None, :])
    d_up = 128 + k[None, :] - k[:, None]  # out row 128+m, src row k
    d_dn = 128 + k[:, None] - k[None, :]  # out row m, src row 128+k
    w = np.zeros((3, P, P), dtype=np.float64)
    for i, dd in enumerate([d_diag, d_up, d_dn]):
        e = -2.0 * dd.astype(np.float64)
        w[i] = np.where(e >= -126, np.exp2(e), 0.0)
    return w.astype(ml_dtypes.bfloat16)


def build_boundary_loss_core(tc, stats_out, logits_in, targets_in, wband_in,
                             use_softplus=False):
    """Emit the per-core kernel. DRAM APs: stats_out [P,8] f32,
    logits_in/targets_in [H,W] f32, wband_in [3,P,P] bf16."""
    nc = tc.nc
    Alu = mybir.AluOpType
    Act = mybir.ActivationFunctionType
    WP = W + 2 * GAP  # padded parabola row width
    WR = WP - 8  # parabola op region width (reads stay in bounds for |d|<=4)

    with (
        tc.tile_pool(name="consts", bufs=1) as consts,
        tc.tile_pool(name="work", bufs=1) as work,
        tc.tile_pool(name="psum", bufs=4, space=bass.MemorySpace.PSUM) as psum,
    ):
        bias0 = consts.tile([P, 1], F32)
        nc.gpsimd.memset(bias0, 0.0)
        bias1 = consts.tile([P, 1], F32)
        nc.gpsimd.memset(bias1, 1.0)
        bias_d2 = {}
        for d in (2, 4):
            bt = consts.tile([P, 1], F32, name=f"bias_d2_{d}")
            nc.gpsimd.memset(bt, float(d * d))
            bias_d2[d] = bt

        # ---- load inputs in row layout [p, g, j] = img[g*128+p, j] ----
        t_src = targets_in.rearrange("(g p) w -> p g w", p=P)
        l_src = logits_in.rearrange("(g p) w -> p g w", p=P)
        wb = consts.tile([P, 3, P], BF16)
        nc.sync.dma_start(out=wb, in_=wband_in.rearrange("b k m -> k b m"))
        t_b = work.tile([P, 2, W], F32)
        nc.gpsimd.dma_start(out=t_b[:, 0], in_=t_src[:, 0])
        nc.scalar.dma_start(out=t_b[:, 1], in_=t_src[:, 1])
        l_b = work.tile([P, 2, W], F32)
        nc.sync.dma_start(out=l_b[:, 0], in_=l_src[:, 0])
        nc.gpsimd.dma_start(out=l_b[:, 1], in_=l_src[:, 1])

        # ---- masks in bf16: m=0 positives (t), m=1 negatives (1-t) ----
        t_bf = work.tile([P, 2, W], BF16)
        nc.vector.tensor_copy(t_bf, t_b)
        nt_bf = work.tile([P, 2, W], BF16)
        nc.vector.tensor_scalar(nt_bf, t_b, -1.0, 1.0, op0=Alu.mult, op1=Alu.add)

        # ---- vertical pass: band matmul + exponent extraction ----
        g2b = work.tile([P, 2, 2, WP], BF16)  # [p, m, g, GAP+j]
        nc.gpsimd.memset(g2b[:, :, :, 0:GAP], BIG)
        nc.gpsimd.memset(g2b[:, :, :, GAP + W:], BIG)
        for m, src in ((0, t_bf), (1, nt_bf)):
            s_ps = psum.tile([P, 2, W], F32, name=f"s_ps{m}")
            nc.tensor.matmul(s_ps[:, 0], wb[:, 0], src[:, 0], start=True, stop=False)
            nc.tensor.matmul(s_ps[:, 0], wb[:, 2], src[:, 1], start=False, stop=True)
            nc.tensor.matmul(s_ps[:, 1], wb[:, 1], src[:, 0], start=True, stop=False)
            nc.tensor.matmul(s_ps[:, 1], wb[:, 0], src[:, 1], start=False, stop=True)
            e32 = work.tile([P, 2, W], U32, name=f"e32_{m}")
            nc.vector.tensor_scalar(
                e32, s_ps.bitcast(U32), 23, None, op0=Alu.logical_shift_right)
            me = work.tile([P, 2, W], U32, name=f"me_{m}")
            nc.vector.tensor_scalar(
                me, e32, -1.0, 128.0, op0=Alu.mult, op1=Alu.add)
            dd = work.tile([P, 2, W], U32, name=f"dd_{m}")
            nc.vector.tensor_scalar(
                dd, me, 1, None, op0=Alu.logical_shift_right)
            nc.vector.tensor_tensor(
                g2b[:, m, :, GAP:GAP + W], dd, dd, Alu.mult)

        # ---- windowed parabola pass along columns ----
        def sh(d):
            return g2b[:, :, :, 4 + d:4 + d + WR]

        acc = work.tile([P, 2, 2, WR], BF16)
        for d in range(1, K + 1):
            u = work.tile([P, 2, 2, WR], BF16, name=f"u{d}")
            nc.vector.tensor_tensor(u, sh(-d), sh(d), Alu.min)
            if d in bias_d2:
                nc.scalar.activation(u, u, Act.Identity, bias=bias_d2[d])
            else:
                nc.vector.tensor_scalar(u, u, float(d * d), None, op0=Alu.add)
            if d == 1:
                nc.vector.tensor_tensor(acc, sh(0), u, Alu.min)
            else:
                nc.vector.tensor_tensor(acc, acc, u, Alu.min)

        # interior view: image col j lives at acc[..., 4 + j]
        w2 = acc[:, :, :, 4:4 + W]  # [p, m, g, j]

        stats = work.tile([P, 8], F32)
        nc.vector.memset(stats, 0.0)
        nc.vector.tensor_reduce(
            stats[:, S_MAXW2:S_MAXW2 + 1], w2, axis=mybir.AxisListType.XYZ,
            op=Alu.max)

        # ---- distances and loss terms ----
        dst = work.tile([P, 2, 2, W], F32)  # [p, m, g, j]; m=0 -> d1, m=1 -> d0
        nc.scalar.activation(dst, w2, Act.Sqrt, bias=bias0)

        sig = work.tile([P, 2, W], F32)
        nc.scalar.activation(
            sig, l_b, Act.Sigmoid, bias=bias0, accum_out=stats[:, S_SIG:S_SIG + 1])
        sp = work.tile([P, 2, W], F32)
        if use_softplus:
            nc.scalar.activation(
                sp, l_b, Act.Softplus, bias=bias0,
                accum_out=stats[:, S_SP:S_SP + 1])
        else:  # softplus = ln(1 + exp(l))
            ex = work.tile([P, 2, W], F32)
            nc.scalar.activation(ex, l_b, Act.Exp, bias=bias0)
            nc.scalar.activation(
                sp, ex, Act.Ln, bias=bias1, accum_out=stats[:, S_SP:S_SP + 1])

        tsum = work.tile([P, 2, W], F32)
        nc.vector.tensor_scalar(
            tsum, t_b, 1.0, None, op0=Alu.mult, op1=Alu.add,
            accum_out=stats[:, S_T:S_T + 1])

        lt = work.tile([P, 2, W], F32)
        nc.vector.scalar_tensor_tensor(
            lt, l_b, 1.0, t_b, op0=Alu.mult, op1=Alu.mult,
            accum_out=stats[:, S_LT:S_LT + 1])
        st = work.tile([P, 2, W], F32)
        nc.vector.scalar_tensor_tensor(
            st, sig, 1.0, t_b, op0=Alu.mult, op1=Alu.mult,
            accum_out=stats[:, S_ST:S_ST + 1])
        sd1 = work.tile([P, 2, W], F32)
        nc.vector.scalar_tensor_tensor(
            sd1, sig, 1.0, dst[:, 0], op0=Alu.mult, op1=Alu.mult,
            accum_out=stats[:, S_SD1:S_SD1 + 1])
        sd0 = work.tile([P, 2, W], F32)
        nc.vector.scalar_tensor_tensor(
            sd0, sig, 1.0, dst[:, 1], op0=Alu.mult, op1=Alu.mult,
            accum_out=stats[:, S_SD0:S_SD0 + 1])

        nc.sync.dma_start(out=stats_out, in_=stats)


_CACHE = {}


def _get_nc():
    if "nc" not in _CACHE:
        nc = bacc.Bacc("TRN2", target_bir_lowering=False, debug=False)
        logits_in = nc.dram_tensor("logits", (H, W), F32, kind="ExternalInput").ap()
        targets_in = nc.dram_tensor("targets", (H, W), F32, kind="ExternalInput").ap()
        wband_in = nc.dram_tensor("wband", (3, P, P), BF16, kind="ExternalInput").ap()
        stats_out = nc.dram_tensor("stats", (P, 8), F32, kind="ExternalOutput").ap()
        with tile.TileContext(nc) as tc:
            build_boundary_loss_core(tc, stats_out, logits_in, targets_in, wband_in)
        nc.compile()
        _CACHE["nc"] = nc
    return _CACHE["nc"]


def combine_stats(stats):
    """stats: (NCORES, P, 8) -> scalar loss (np.float32). None if the
    windowed EDT was not provably exact (caller must fall back)."""
    maxw2 = float(stats[:, :, S_MAXW2].max())
    if maxw2 > K * K + 0.5:
        return None
    s = stats.sum(axis=1, dtype=np.float64)  # (NCORES, 8)
    n = float(B * H * W)
    s_sig, s_t = s[:, S_SIG], s[:, S_T]
    s_lt, s_st = s[:, S_LT], s[:, S_ST]
    s_sp = s[:, S_SP]
    s_sdq = s[:, S_SD1] - s[:, S_SD0]
    has_pos = s_t > 0
    inter = s_st.sum()
    union = s_sig.sum() + s_t.sum() + SMOOTH
    dice = 1.0 - (2.0 * inter + SMOOTH) / union
    bce = (s_sp.sum() - s_lt.sum()) / n
    bdy = np.where(has_pos, s_sdq + s_st, 0.0).sum() / n
    return np.float32(0.5 * dice + 0.5 * bce + 0.5 * bdy)


def run_device(logits, targets, trace=False, trace_cores=None):
    l = np.ascontiguousarray(np.asarray(logits, np.float32).reshape(NCORES, H, W))
    t = np.ascontiguousarray(np.asarray(targets, np.float32).reshape(NCORES, H, W))
    wband = make_wband()
    in_maps = [
        {"logits": l[i], "targets": t[i], "wband": wband} for i in range(NCORES)
    ]
    nc = _get_nc()
    res = run_bass_kernel_spmd(
        nc, in_maps, core_ids=list(range(NCORES)), trace=trace,
        trace_cores=trace_cores)
    stats = np.stack([res.results[i]["stats"] for i in range(NCORES)])
    return stats, res


# ---------------- host fallback (exact reference semantics) ----------------

def _edt_np(mask):
    """Exact EDT (distance to nearest True) matching the reference."""
    h, w = mask.shape
    big = float(h * w)
    c = np.where(mask, 0.0, np.inf)
    f = np.empty((h, w))
    s = np.full((w,), big)
    for i in range(h):
        s = np.minimum(s + 1.0, c[i])
        f[i] = s
    g = np.empty((h, w))
    s = np.full((w,), big)
    for i in reversed(range(h)):
        s = np.minimum(s + 1.0, f[i])
        g[i] = s
    g2 = g * g
    jj = np.arange(w, dtype=np.float64)
    dj2 = (jj[:, None] - jj[None, :]) ** 2  # (j_out, j_src)
    d2 = np.empty((h, w))
    for i in range(h):
        d2[i] = (g2[i][None, :] + dj2).min(axis=1)
    return np.sqrt(d2)


def _fallback_loss(logits, targets):
    l = np.asarray(logits, np.float64).reshape(B, H, W)
    t = np.asarray(targets, np.float64).reshape(B, H, W)
    sig = 1.0 / (1.0 + np.exp(-l))
    inter = (sig * t).sum()
    union = sig.sum() + t.sum() + SMOOTH
    dice = 1.0 - (2.0 * inter + SMOOTH) / union
    bce = (np.logaddexp(l, 0.0) - l * t).mean()
    bdy_sum = 0.0
    for b_i in range(B):
        m = t[b_i] > 0.5
        if not m.any():
            continue
        d1 = _edt_np(m)
        d0 = _edt_np(~m)
        res = d1 * (1.0 - t[b_i]) - (d0 - 1.0) * t[b_i]
        bdy_sum += (sig[b_i] * res).sum()
    bdy = bdy_sum / float(B * H * W)
    return np.float32(0.5 * dice + 0.5 * bce + 0.5 * bdy)


def kernel(logits, targets):
    stats, _ = run_device(logits, targets)
    loss = combine_stats(stats)
    if loss is None:
        loss = _fallback_loss(logits, targets)
    return np.array(loss, dtype=np.float32)
